# revision 1
# baseline (speedup 1.0000x reference)
# Differential GQA attention layer (B=2, S=1024, E=2048, H=16, KVH=4, D=128)
# distributed over 8 TRN2 NeuronCores: shard = (batch b, kv-group g) so each
# core owns 1 batch x 4 query heads (1 kv head). All attention is core-local;
# the Wo row-sharded output projection partials are summed on the host.
#
# Self-contained: hardcodes shapes/sharding; builds+compiles a Bass/Tile
# kernel on first call and runs it via run_bass_kernel_spmd on cores 0-7.
import numpy as np

B, S, E, H, KVH = 2, 1024, 2048, 16, 4
D = 128
NEG = -1e30
LAM_INIT = 0.2  # 0.8 - 0.6*exp(-0.3*layer_idx), layer_idx=0
NCORES = 8
HPC = H // KVH  # heads per core = 4

MM_DT = "bf16"  # PE path dtype: bf16 LDWEIGHTS+MM is ~1.9x faster than f32/f32r

_cache = {}


def _build(dbg=False):
    import concourse.mybir as mybir
    import concourse.tile as tile
    from concourse import bacc
    from concourse.masks import make_identity
    from contextlib import ExitStack

    F32 = mybir.dt.float32
    BF16 = mybir.dt.bfloat16
    MMD = BF16 if MM_DT == "bf16" else (mybir.dt.float32r if MM_DT == "f32r" else F32)
    ALU = mybir.AluOpType
    ACT = mybir.ActivationFunctionType

    nc = bacc.Bacc(None, target_bir_lowering=False)

    xT = nc.declare_dram_parameter("xT", [E, S], MMD, isOutput=False)
    Wq = nc.declare_dram_parameter("Wq", [E, HPC * 2 * D], MMD, isOutput=False)
    Wk = nc.declare_dram_parameter("Wk", [E, 2 * D], MMD, isOutput=False)
    Wv = nc.declare_dram_parameter("Wv", [E, D], MMD, isOutput=False)
    Wo = nc.declare_dram_parameter("Wo", [HPC * D, E], MMD, isOutput=False)
    cosd = nc.declare_dram_parameter("cosd", [2 * D, S], F32, isOutput=False)
    sind = nc.declare_dram_parameter("sind", [2 * D, S], F32, isOutput=False)
    lamn = nc.declare_dram_parameter("lamn", [D, HPC], F32, isOutput=False)
    maskn = nc.declare_dram_parameter("maskn", [D, D], F32, isOutput=False)
    out_ext = nc.declare_dram_parameter("out", [S, E], F32, isOutput=True)
    if dbg:
        BF = mybir.dt.bfloat16
        dq = nc.declare_dram_parameter("dq", [128, S], F32, isOutput=True)
        dk = nc.declare_dram_parameter("dk", [128, S], F32, isOutput=True)
        dv = nc.declare_dram_parameter("dv", [128, S], F32, isOutput=True)
        dvt = nc.declare_dram_parameter("dvt", [128, 128], BF, isOutput=True)
        dexp2 = nc.declare_dram_parameter("dexp2", [128, S], BF, isOutput=True)
        ddifft = nc.declare_dram_parameter("ddifft", [128, S], BF, isOutput=True)
        dattf = nc.declare_dram_parameter("dattf", [128, 512], F32, isOutput=True)

    ISCALE = 1.0 / float(np.sqrt(D))
    NQT = S // 128
    NPC = S // 512

    with tile.TileContext(nc) as tc:
        with ExitStack() as ctx:
            cpool = ctx.enter_context(tc.tile_pool(name="const", bufs=1))
            qkpool = ctx.enter_context(tc.tile_pool(name="qk", bufs=1))
            smalls = ctx.enter_context(tc.tile_pool(name="smalls", bufs=2))

            # constants (tiles now; DMAs deferred until after the first
            # m1 block so xT/W loads own the queues at kernel start)
            cos_t = [cpool.tile([128, S], F32, tag=f"cos{a}", name=f"cos{a}") for a in range(2)]
            sin_t = [cpool.tile([128, S], F32, tag=f"sin{a}", name=f"sin{a}") for a in range(2)]
            lam_t = cpool.tile([128, HPC], F32, tag="lam", name="lam")
            mask_t = cpool.tile([128, 128], F32, tag="mask", name="mask")

            def load_consts():
                for a in range(2):
                    nc.sync.dma_start(out=cos_t[a][:], in_=cosd[a * 128:(a + 1) * 128, :])
                    nc.sync.dma_start(out=sin_t[a][:], in_=sind[a * 128:(a + 1) * 128, :])
                nc.sync.dma_start(out=lam_t[:], in_=lamn[:])
                nc.sync.dma_start(out=mask_t[:], in_=maskn[:])
            ident = cpool.tile([128, 128], F32, tag="ident", name="ident")
            make_identity(nc, ident[:])
            identb = cpool.tile([128, 128], BF16, tag="identb", name="identb")
            make_identity(nc, identb[:])

            # persistent activations
            qT = [[qkpool.tile([128, S], MMD, tag=f"qT{h}{a}", name=f"qT{h}{a}")
                   for a in range(2)] for h in range(HPC)]
            kT = [qkpool.tile([128, S], MMD, tag=f"kT{a}", name=f"kT{a}") for a in range(2)]
            vT = qkpool.tile([128, S], MMD, tag="vT", name="vT")
            v_t = [qkpool.tile([128, 128], BF16, tag=f"v{j}", name=f"v{j}")
                   for j in range(NQT)]
            attf = [[qkpool.tile([128, 512], MMD, tag=f"attf{h}{c}", name=f"attf{h}{c}")
                     for c in range(NPC)] for h in range(HPC)]

            # ------------- phase A: x @ W -> qT/kT/vT (+rope), v -------------
            with ExitStack() as actx:
                xpool = actx.enter_context(tc.tile_pool(name="xT", bufs=16))
                wpool = actx.enter_context(tc.tile_pool(name="w", bufs=6))
                m1ps = actx.enter_context(tc.tile_pool(name="m1ps", bufs=6, space="PSUM"))
                vtrps = actx.enter_context(tc.tile_pool(name="vtrps", bufs=2, space="PSUM"))
                rtmp = actx.enter_context(tc.tile_pool(name="rtmp", bufs=3))

                # k and v first so phase B can start while q streams
                blocks = []
                for a in range(2):
                    blocks.append(("k", Wk, a * 128, None, a))
                blocks.append(("v", Wv, 0, None, None))
                for h in range(HPC):
                    for a in range(2):
                        blocks.append(("q", Wq, (h * 2 + a) * 128, h, a))

                xt = [None] * 16
                for bi, (kind, wsrc, c0, h, a) in enumerate(blocks):
                    pst = [m1ps.tile([128, 512], F32, tag="m1", name="m1")
                           for _ in range(NPC)]
                    for e in range(16):
                        if bi == 0:
                            # lazy xT load interleaved with first block's weights
                            t = xpool.tile([128, S], MMD, tag="xt", name="xt")
                            nc.sync.dma_start(
                                out=t[:], in_=xT[e * 128:(e + 1) * 128, :])
                            xt[e] = t
                        wt = wpool.tile([128, 128], MMD, tag="w", name="w")
                        nc.sync.dma_start(
                            out=wt[:],
                            in_=wsrc[e * 128:(e + 1) * 128, c0:c0 + 128])
                        for p in range(NPC):
                            nc.tensor.matmul(
                                pst[p][:], wt[:], xt[e][:, p * 512:(p + 1) * 512],
                                start=(e == 0), stop=(e == 15))
                    if bi == 0:
                        load_consts()
                    for p in range(NPC):
                        ps = pst[p]
                        sl = slice(p * 512, (p + 1) * 512)
                        if kind == "v":
                            nc.scalar.copy(vT[:, sl], ps[:])
                        else:
                            dst = qT[h][a] if kind == "q" else kT[a]
                            tmp = rtmp.tile([128, 512], F32, tag="swap", name="swap")
                            nc.vector.tensor_copy(tmp[0:64, :], ps[64:128, :])
                            nc.vector.tensor_copy(tmp[64:128, :], ps[0:64, :])
                            nc.vector.tensor_tensor(
                                dst[:, sl], ps[:], cos_t[a][:, sl], op=ALU.mult)
                            nc.vector.tensor_tensor(
                                tmp[:], tmp[:], sin_t[a][:, sl], op=ALU.mult)
                            nc.vector.tensor_tensor(
                                dst[:, sl], dst[:, sl], tmp[:], op=ALU.add)

                # v: [d, pos] -> v_t[j]: [128 pos, 128 d] (bf16 for PV)
                vps = [vtrps.tile([128, 128], BF16, tag="vtr", name="vtr")
                       for j in range(NQT)]
                for j in range(NQT):
                    nc.tensor.transpose(
                        vps[j][:], vT[:, j * 128:(j + 1) * 128], identb[:])
                    nc.scalar.copy(v_t[j][:], vps[j][:])

            if dbg:
                nc.sync.dma_start(out=dq[:], in_=qT[0][0][:].bitcast(F32))
                nc.sync.dma_start(out=dk[:], in_=kT[0][:].bitcast(F32))
                nc.sync.dma_start(out=dv[:], in_=vT[:])
                nc.sync.dma_start(out=dvt[:], in_=v_t[0][:])

            # ------------- phase B: attention per head -------------
            with ExitStack() as bctx:
                eps = bctx.enter_context(tc.tile_pool(name="eps", bufs=5, space="PSUM"))
                epool = bctx.enter_context(tc.tile_pool(name="expp", bufs=3))
                dtpool = bctx.enter_context(tc.tile_pool(name="difft", bufs=24))
                pvtr = bctx.enter_context(tc.tile_pool(name="pvtr", bufs=3, space="PSUM"))
                sm2 = bctx.enter_context(tc.tile_pool(name="sm2", bufs=2))

                for h in range(HPC):
                    diffT = [dtpool.tile([128, S], BF16, tag="difft", name="difft")
                             for _ in range(NQT)]

                    for i in range(NQT):
                        Ke = (i + 1) * 128
                        nch = 1 if Ke <= 512 else 2
                        # energy psum: chunk tiles (a: k<512, b: k>=512)
                        e0c = [eps.tile([128, 512], F32, tag="e", name="e0a")]
                        e1c = [eps.tile([128, 512], F32, tag="e", name="e1a")]
                        if nch == 2:
                            e0c.append(eps.tile([128, 512], F32, tag="e", name="e0b"))
                            e1c.append(eps.tile([128, 512], F32, tag="e", name="e1b"))
                        for kc in range(nch):
                            w = min(Ke, (kc + 1) * 512) - kc * 512
                            ksl = slice(kc * 512, kc * 512 + w)
                            nc.tensor.matmul(
                                e0c[kc][:, 0:w],
                                qT[h][0][:, i * 128:(i + 1) * 128],
                                kT[0][:, ksl], start=True, stop=True)
                            nc.tensor.matmul(
                                e1c[kc][:, 0:w],
                                qT[h][1][:, i * 128:(i + 1) * 128],
                                kT[1][:, ksl], start=True, stop=True)
                        # causal mask on the diagonal 128-block
                        dc, doff = (i * 128) // 512, (i * 128) % 512
                        dw = slice(doff, doff + 128)
                        nc.vector.tensor_tensor(e0c[dc][:, dw], e0c[dc][:, dw],
                                                mask_t[:], op=ALU.add)
                        nc.vector.tensor_tensor(e1c[dc][:, dw], e1c[dc][:, dw],
                                                mask_t[:], op=ALU.add)

                        exp0 = epool.tile([128, S], F32, tag="exp0", name="exp0")
                        exp1 = epool.tile([128, S], F32, tag="exp1", name="exp1")
                        s0p = [sm2.tile([128, 1], F32, tag="s0p", name="s0p", bufs=4)
                               for _ in range(nch)]
                        s1p = [sm2.tile([128, 1], F32, tag="s1p", name="s1p", bufs=4)
                               for _ in range(nch)]
                        for kc in range(nch):
                            w = min(Ke, (kc + 1) * 512) - kc * 512
                            osl = slice(kc * 512, kc * 512 + w)
                            nc.scalar.activation(exp0[:, osl], e0c[kc][:, 0:w], ACT.Exp,
                                                 scale=ISCALE, accum_out=s0p[kc][:])
                            nc.scalar.activation(exp1[:, osl], e1c[kc][:, 0:w], ACT.Exp,
                                                 scale=ISCALE, accum_out=s1p[kc][:])
                        s0 = sm2.tile([128, 1], F32, tag="s0", name="s0")
                        s1 = sm2.tile([128, 1], F32, tag="s1", name="s1")
                        if nch == 2:
                            nc.vector.tensor_tensor(s0[:], s0p[0][:], s0p[1][:], op=ALU.add)
                            nc.vector.tensor_tensor(s1[:], s1p[0][:], s1p[1][:], op=ALU.add)
                        else:
                            s0, s1 = s0p[0], s1p[0]
                        r0 = sm2.tile([128, 1], F32, tag="r0", name="r0")
                        r1 = sm2.tile([128, 1], F32, tag="r1", name="r1")
                        r1p = sm2.tile([128, 1], F32, tag="r1p", name="r1p")
                        nc.vector.reciprocal(r0[:], s0[:])
                        nc.vector.reciprocal(r1[:], s1[:])
                        nc.vector.scalar_tensor_tensor(
                            r1p[:], s0[:], lam_t[:, h:h + 1], r1[:],
                            op0=ALU.mult, op1=ALU.mult)
                        # t = (exp1 * r1p) + exp0 ; diag-masked; exp2 = Exp(r0*t)
                        t = epool.tile([128, S], F32, tag="t", name="t")
                        nc.vector.scalar_tensor_tensor(
                            t[:, :Ke], exp1[:, :Ke], r1p[:], exp0[:, :Ke],
                            op0=ALU.mult, op1=ALU.add)
                        nc.vector.tensor_tensor(t[:, i * 128:(i + 1) * 128],
                                                t[:, i * 128:(i + 1) * 128],
                                                mask_t[:], op=ALU.add)
                        exp2 = epool.tile([128, S], BF16, tag="exp2", name="exp2")
                        nc.scalar.activation(exp2[:, :Ke], t[:, :Ke], ACT.Exp,
                                             scale=r0[:])
                        if dbg and h == 0 and i == NQT - 1:
                            nc.sync.dma_start(out=dexp2[:], in_=exp2[:])
                        # transpose scores into diffT column blocks (PE, bf16)
                        for j in range(i + 1):
                            tp = pvtr.tile([128, 512], BF16, tag="pvtr", name="tr")
                            nc.tensor.transpose(
                                tp[:, 0:128], exp2[:, j * 128:(j + 1) * 128],
                                identb[:])
                            nc.vector.tensor_copy(
                                diffT[j][:, i * 128:(i + 1) * 128], tp[:, 0:128])

                    if dbg and h == 0:
                        nc.sync.dma_start(out=ddifft[:], in_=diffT[0][:])
                    # PV + RMS normalization per 512-q chunk
                    # att_final = att_raw * sqrt(128/ss); softmax2 norm cancels
                    for c in range(NPC):
                        nk = 4 * c + 4
                        attps = pvtr.tile([128, 512], F32, tag="pvtr", name="att")
                        for j in range(nk):
                            off = max(0, j * 128 - c * 512)
                            nc.tensor.matmul(
                                attps[:, off:512], v_t[j][:],
                                diffT[j][:, c * 512 + off:(c + 1) * 512],
                                start=(j == 0), stop=(j == nk - 1))
                        att2 = sm2.tile([128, 512], F32, tag="att2", name="att2")
                        nc.scalar.square(att2[:], attps[:])
                        sssb = sm2.tile([1, 512], F32, tag="sssb", name="sssb")
                        nc.gpsimd.tensor_reduce(sssb[:], att2[:],
                                                axis=mybir.AxisListType.C,
                                                op=ALU.add)
                        w = sm2.tile([1, 512], F32, tag="w", name="w")
                        nc.vector.tensor_scalar(
                            w[:], sssb[:], 1.0 / 128.0, 1e-30,
                            op0=ALU.mult, op1=ALU.add)
                        rci = sm2.tile([1, 512], F32, tag="rci", name="rci")
                        nc.vector.reciprocal(rci[:], w[:])
                        cfac = sm2.tile([1, 512], F32, tag="cfac", name="cfac")
                        nc.scalar.sqrt(cfac[:], rci[:])
                        bsb = sm2.tile([128, 512], F32, tag="bsb", name="bsb")
                        nc.gpsimd.partition_broadcast(bsb[:], cfac[:])
                        nc.vector.tensor_tensor(
                            attf[h][c][:], attps[:], bsb[:], op=ALU.mult)
                        if dbg and h == 0 and c == 0:
                            nc.sync.dma_start(out=dattf[:], in_=attf[0][0][:].bitcast(F32))

            # ------------- phase C: out = attf^T @ Wo -------------
            with ExitStack() as cctx:
                wops = cctx.enter_context(tc.tile_pool(name="wops", bufs=4, space="PSUM"))
                opool = cctx.enter_context(tc.tile_pool(name="osb", bufs=4))
                wopool = cctx.enter_context(tc.tile_pool(name="wop", bufs=1))
                wo_t = [wopool.tile([128, E], MMD, tag=f"wo{h}", name=f"wo{h}")
                        for h in range(HPC)]
                for h in range(HPC):
                    nc.sync.dma_start(out=wo_t[h][:],
                                      in_=Wo[h * 128:(h + 1) * 128, :])
                for p in range(NQT):
                    c, po = p // 4, (p % 4) * 128
                    for n in range(E // 512):
                        ops = wops.tile([128, 512], F32, tag="o", name="o")
                        for h in range(HPC):
                            nc.tensor.matmul(
                                ops[:], attf[h][c][:, po:po + 128],
                                wo_t[h][:, n * 512:(n + 1) * 512],
                                start=(h == 0), stop=(h == HPC - 1))
                        osb = opool.tile([128, 512], F32, tag="osb", name="osb")
                        nc.scalar.copy(osb[:], ops[:])
                        nc.sync.dma_start(
                            out=out_ext[p * 128:(p + 1) * 128, n * 512:(n + 1) * 512],
                            in_=osb[:])

    nc.finalize()
    return nc


def _host_prep(x, Wq, Wk, Wv, Wo, lq1, lq2, lk1, lk2, rms_w):
    lam = (np.exp((lq1 * lk1).sum(-1)) - np.exp((lq2 * lk2).sum(-1))
           + LAM_INIT).astype(np.float32)  # (H,)
    j = np.arange(D, dtype=np.float64)
    theta = 1.0 / (10000.0 ** (2.0 * j / (2 * D)))
    pos = np.arange(S, dtype=np.float64)
    ang = pos[None, :] * theta[:, None]  # (128, S)
    cosd = np.cos(ang).astype(np.float32)
    sin = np.sin(ang)
    cosd2 = np.concatenate([np.concatenate([cosd[a * 64:(a + 1) * 64]] * 2, 0)
                            for a in range(2)], 0)
    sind2 = np.concatenate(
        [np.concatenate([-sin[a * 64:(a + 1) * 64], sin[a * 64:(a + 1) * 64]], 0)
         for a in range(2)], 0).astype(np.float32)

    perm256 = np.concatenate([np.arange(0, 128, 2), np.arange(1, 128, 2),
                              np.arange(128, 256, 2), np.arange(129, 256, 2)])
    Wqp = Wq.reshape(E, H, 2 * D)[:, :, perm256].reshape(E, H * 2 * D)
    Wkp = Wk.reshape(E, KVH, 2 * D)[:, :, perm256].reshape(E, KVH * 2 * D)
    WoS = (Wo.reshape(H, D, E) * (rms_w[None, :, None] * (1.0 - LAM_INIT))
           ).reshape(E, E).astype(np.float32)

    maskn = np.where(np.arange(128)[None, :] > np.arange(128)[:, None],
                     np.float32(NEG), np.float32(0.0)).astype(np.float32)

    import ml_dtypes
    bf = ml_dtypes.bfloat16
    in_maps = []
    for core in range(NCORES):
        b, g = divmod(core, KVH)
        heads = slice(HPC * g * 2 * D, HPC * (g + 1) * 2 * D)
        lam_g = lam[HPC * g:HPC * (g + 1)]
        in_maps.append({
            "xT": np.ascontiguousarray(x[b].T).astype(bf),
            "Wq": np.ascontiguousarray(Wqp[:, heads]).astype(bf),
            "Wk": np.ascontiguousarray(Wkp[:, g * 2 * D:(g + 1) * 2 * D]).astype(bf),
            "Wv": np.ascontiguousarray(Wv[:, g * D:(g + 1) * D]).astype(bf),
            "Wo": np.ascontiguousarray(WoS[HPC * D * g:HPC * D * (g + 1), :]).astype(bf),
            "cosd": cosd2,
            "sind": sind2,
            "lamn": np.tile(-lam_g[None, :], (D, 1)).astype(np.float32),
            "maskn": maskn,
        })
    return in_maps


def kernel(x, Wq, Wk, Wv, Wo, lq1, lq2, lk1, lk2, rms_w, _trace=False):
    from concourse import bass_utils

    in_maps = _host_prep(np.asarray(x, np.float32), np.asarray(Wq, np.float32),
                         np.asarray(Wk, np.float32), np.asarray(Wv, np.float32),
                         np.asarray(Wo, np.float32), np.asarray(lq1, np.float32),
                         np.asarray(lq2, np.float32), np.asarray(lk1, np.float32),
                         np.asarray(lk2, np.float32), np.asarray(rms_w, np.float32))
    if "nc" not in _cache:
        _cache["nc"] = _build()
    nc = _cache["nc"]
    res = bass_utils.run_bass_kernel_spmd(
        nc, in_maps, core_ids=list(range(NCORES)), trace=_trace)
    _cache["last_result"] = res
    parts = np.stack([res.results[c]["out"] for c in range(NCORES)], 0)
    out = parts.reshape(B, KVH, S, E).sum(1)
    return out.astype(np.float32)



# revision 14
# speedup vs baseline: 3.5296x; 3.5296x over previous
# Differential GQA attention layer (B=2, S=1024, E=2048, H=16, KVH=4, D=128)
# distributed over 8 TRN2 NeuronCores: shard = (batch b, kv-group g) so each
# core owns 1 batch x 4 query heads (1 kv head). All attention is core-local;
# the Wo row-sharded output projection partials are summed on the host.
#
# Self-contained: hardcodes shapes/sharding; builds+compiles a Bass/Tile
# kernel on first call and runs it via run_bass_kernel_spmd on cores 0-7.
import numpy as np

B, S, E, H, KVH = 2, 1024, 2048, 16, 4
D = 128
NEG = -1e30
LAM_INIT = 0.2  # 0.8 - 0.6*exp(-0.3*layer_idx), layer_idx=0
NCORES = 8
HPC = H // KVH  # heads per core = 4

_cache = {}


def _build():
    import concourse.mybir as mybir
    import concourse.tile as tile
    from concourse import bacc
    from concourse.masks import make_identity
    from contextlib import ExitStack

    F32 = mybir.dt.float32
    F32R = mybir.dt.float32r
    BF16 = mybir.dt.bfloat16
    ALU = mybir.AluOpType
    ACT = mybir.ActivationFunctionType

    nc = bacc.Bacc(None, target_bir_lowering=False)

    xT = nc.declare_dram_parameter("xT", [E, S], BF16, isOutput=False)
    Wq = nc.declare_dram_parameter("Wq", [E, HPC * 2 * D], BF16, isOutput=False)
    Wk = nc.declare_dram_parameter("Wk", [E, 2 * D], BF16, isOutput=False)
    Wv = nc.declare_dram_parameter("Wv", [E, D], BF16, isOutput=False)
    Wo = nc.declare_dram_parameter("Wo", [HPC * D, E], BF16, isOutput=False)
    cosd = nc.declare_dram_parameter("cosd", [2 * D, S], BF16, isOutput=False)
    sind = nc.declare_dram_parameter("sind", [2 * D, S], BF16, isOutput=False)
    lamn = nc.declare_dram_parameter("lamn", [D, HPC], F32, isOutput=False)
    maskn = nc.declare_dram_parameter("maskn", [D, D], BF16, isOutput=False)
    mask01 = nc.declare_dram_parameter("mask01", [D, D], BF16, isOutput=False)
    out_ext = nc.declare_dram_parameter("out", [S, E], BF16, isOutput=True)

    ISCALE = 1.0 / float(np.sqrt(D))
    NQT = S // 128
    NPC = S // 512

    with tile.TileContext(nc) as tc:
        with ExitStack() as ctx:
            cpool = ctx.enter_context(tc.tile_pool(name="const", bufs=1))
            wpool = ctx.enter_context(tc.tile_pool(name="wts", bufs=1))
            qkpool = ctx.enter_context(tc.tile_pool(name="qk", bufs=1))
            smalls = ctx.enter_context(tc.tile_pool(name="smalls", bufs=2))

            # ---- weights + x resident in SBUF; full-width rows => 2KB DMA lines
            wk_t = [wpool.tile([128, 2 * D], BF16, tag=f"wk{e}", name=f"wk{e}")
                    for e in range(16)]
            xt = [wpool.tile([128, S], BF16, tag=f"xt{e}", name=f"xt{e}")
                  for e in range(16)]
            wv_t = [wpool.tile([128, D], BF16, tag=f"wv{e}", name=f"wv{e}")
                    for e in range(16)]
            wq_t = [wpool.tile([128, HPC * 2 * D], BF16, tag=f"wq{e}", name=f"wq{e}")
                    for e in range(16)]
            wo_t = [wpool.tile([128, E], BF16, tag=f"wo{h}", name=f"wo{h}")
                    for h in range(HPC)]
            for e in range(16):
                nc.sync.dma_start(out=wk_t[e][:], in_=Wk[e * 128:(e + 1) * 128, :])
                nc.sync.dma_start(out=xt[e][:], in_=xT[e * 128:(e + 1) * 128, :])
            for e in range(16):
                nc.sync.dma_start(out=wv_t[e][:], in_=Wv[e * 128:(e + 1) * 128, :])
            for e in range(16):
                nc.sync.dma_start(out=wq_t[e][:], in_=Wq[e * 128:(e + 1) * 128, :])

            # constants
            cos_t = [cpool.tile([128, S], BF16, tag=f"cos{a}", name=f"cos{a}") for a in range(2)]
            sin_t = [cpool.tile([128, S], BF16, tag=f"sin{a}", name=f"sin{a}") for a in range(2)]
            lam_t = cpool.tile([128, HPC], F32, tag="lam", name="lam")
            mask_t = cpool.tile([128, 128], BF16, tag="mask", name="mask")
            m01_t = cpool.tile([128, 128], BF16, tag="m01", name="m01")
            for a in range(2):
                nc.sync.dma_start(out=cos_t[a][:], in_=cosd[a * 128:(a + 1) * 128, :])
                nc.sync.dma_start(out=sin_t[a][:], in_=sind[a * 128:(a + 1) * 128, :])
            nc.sync.dma_start(out=lam_t[:], in_=lamn[:])
            nc.sync.dma_start(out=mask_t[:], in_=maskn[:])
            nc.sync.dma_start(out=m01_t[:], in_=mask01[:])
            for h in range(HPC):
                nc.sync.dma_start(out=wo_t[h][:], in_=Wo[h * 128:(h + 1) * 128, :])

            identb = cpool.tile([128, 128], BF16, tag="identb", name="identb")
            make_identity(nc, identb[:])
            ones_c32 = cpool.tile([128, 1], F32, tag="ones_c32", name="ones_c32")
            nc.vector.memset(ones_c32[:], 1.0)
            ones_c = cpool.tile([128, 1], F32R, tag="ones_c", name="ones_c")
            nc.scalar.copy(ones_c[:], ones_c32[:])
            ones_r32 = cpool.tile([1, 128], F32, tag="ones_r32", name="ones_r32")
            nc.vector.memset(ones_r32[:], 1.0)
            ones_r = cpool.tile([1, 128], F32R, tag="ones_r", name="ones_r")
            nc.scalar.copy(ones_r[:], ones_r32[:])
            eps_t = cpool.tile([1, 1], F32, tag="eps", name="eps")
            nc.vector.memset(eps_t[:], 1e-8)

            # persistent activations
            qT = [[qkpool.tile([128, S], BF16, tag=f"qT{h}{a}", name=f"qT{h}{a}")
                   for a in range(2)] for h in range(HPC)]
            kT = [qkpool.tile([128, S], BF16, tag=f"kT{a}", name=f"kT{a}") for a in range(2)]
            vT = qkpool.tile([128, S], BF16, tag="vT", name="vT")
            v_t = [qkpool.tile([128, 128], BF16, tag=f"v{j}", name=f"v{j}")
                   for j in range(NQT)]
            attf = [[qkpool.tile([128, 512], BF16, tag=f"attf{h}{c}", name=f"attf{h}{c}")
                     for c in range(NPC)] for h in range(HPC)]

            # ------------- phase A: x @ W -> qT/kT/vT (+rope), v -------------
            with ExitStack() as actx:
                m1ps = actx.enter_context(tc.tile_pool(name="m1ps", bufs=4, space="PSUM"))
                vtrps = actx.enter_context(tc.tile_pool(name="vtrps", bufs=2, space="PSUM"))
                rtmp = actx.enter_context(tc.tile_pool(name="rtmp", bufs=3))

                # k first, then v, then q: phase B head h can start once its q done
                blocks = [("k", 0), ("k", 1), ("v", None)]
                for h in range(HPC):
                    for a in range(2):
                        blocks.append(("q", (h, a)))

                for kind, meta in blocks:
                    pst = [m1ps.tile([128, 512], F32, tag="m1", name="m1")
                           for _ in range(NPC)]
                    for e in range(16):
                        if kind == "k":
                            wsl = wk_t[e][:, meta * 128:(meta + 1) * 128]
                        elif kind == "v":
                            wsl = wv_t[e][:]
                        else:
                            h, a = meta
                            c0 = (h * 2 + a) * 128
                            wsl = wq_t[e][:, c0:c0 + 128]
                        for p in range(NPC):
                            nc.tensor.matmul(
                                pst[p][:], wsl, xt[e][:, p * 512:(p + 1) * 512],
                                start=(e == 0), stop=(e == 15))
                    for p in range(NPC):
                        ps = pst[p]
                        sl = slice(p * 512, (p + 1) * 512)
                        if kind == "v":
                            nc.scalar.copy(vT[:, sl], ps[:])
                            continue
                        a = meta if kind == "k" else meta[1]
                        dst = kT[a] if kind == "k" else qT[meta[0]][a]
                        # rope: dst = c*cos + swap(c)*sin. sin is host-swapped
                        # (sinS[p] = sin[swap(p)]) so each TT reads equal input
                        # bases and only the OUT partition base is shifted.
                        c = rtmp.tile([128, 512], BF16, tag="rc", name="rc")
                        nc.scalar.copy(c[:], ps[:])
                        tmp = rtmp.tile([128, 512], BF16, tag="rt", name="rt")
                        nc.vector.tensor_tensor(
                            tmp[0:64, :], c[64:128, :], sin_t[a][64:128, sl], op=ALU.mult)
                        nc.vector.tensor_tensor(
                            tmp[64:128, :], c[0:64, :], sin_t[a][0:64, sl], op=ALU.mult)
                        nc.vector.tensor_tensor(
                            dst[:, sl], c[:], cos_t[a][:, sl], op=ALU.mult)
                        nc.vector.tensor_tensor(
                            dst[:, sl], dst[:, sl], tmp[:], op=ALU.add)

                # v: [d, pos] -> v_t[j]: [128 pos, 128 d] (bf16 for PV)
                for j in range(NQT):
                    vps = vtrps.tile([128, 128], BF16, tag="vtr", name="vtr")
                    nc.tensor.transpose(
                        vps[:], vT[:, j * 128:(j + 1) * 128], identb[:])
                    nc.scalar.copy(v_t[j][:], vps[:])

            # ------------- phase B: attention per head -------------
            with ExitStack() as bctx:
                eps = bctx.enter_context(tc.tile_pool(name="eps", bufs=4, space="PSUM"))
                tpps = bctx.enter_context(tc.tile_pool(name="tpps", bufs=1, space="PSUM"))
                pvps = bctx.enter_context(tc.tile_pool(name="pvps", bufs=1, space="PSUM"))
                rmsps = bctx.enter_context(tc.tile_pool(name="rmsps", bufs=1, space="PSUM"))
                epool = bctx.enter_context(tc.tile_pool(name="expp", bufs=2))
                dtpool = bctx.enter_context(tc.tile_pool(name="difft", bufs=12))
                sm2 = bctx.enter_context(tc.tile_pool(name="sm2", bufs=3))

                for h in range(HPC):
                    diffT = [dtpool.tile([128, S], BF16, tag="difft", name="difft")
                             for _ in range(NQT)]

                    for i in range(NQT):
                        Ke = (i + 1) * 128
                        nch = 1 if Ke <= 512 else 2
                        dc, doff = (i * 128) // 512, (i * 128) % 512
                        ec = [[eps.tile([128, 512], F32, tag="e", name=f"e{half}")
                               for _ in range(nch)] for half in range(2)]
                        # energy matmuls; diag 128-block gets causal mask added
                        # on the PE (identity @ mask accumulated into psum)
                        for half in range(2):
                            for kc in range(nch):
                                w = min(Ke, (kc + 1) * 512) - kc * 512
                                ksl = slice(kc * 512, kc * 512 + w)
                                qsl = qT[h][half][:, i * 128:(i + 1) * 128]
                                if kc == dc:
                                    if doff > 0:
                                        nc.tensor.matmul(
                                            ec[half][kc][:, 0:doff], qsl,
                                            kT[half][:, kc * 512:kc * 512 + doff],
                                            start=True, stop=True)
                                    nc.tensor.matmul(
                                        ec[half][kc][:, doff:doff + 128], qsl,
                                        kT[half][:, i * 128:i * 128 + 128],
                                        start=True, stop=False)
                                else:
                                    nc.tensor.matmul(
                                        ec[half][kc][:, 0:w], qsl,
                                        kT[half][:, ksl], start=True, stop=True)
                        for half in range(2):
                            nc.tensor.matmul(
                                ec[half][dc][:, doff:doff + 128], identb[:],
                                mask_t[:], start=False, stop=True)

                        exp0 = epool.tile([128, S], BF16, tag="exp0", name="exp0")
                        exp1 = epool.tile([128, S], BF16, tag="exp1", name="exp1")
                        sA = sm2.tile([128, 4], F32, tag="sA", name="sA")
                        for kc in range(nch):
                            w = min(Ke, (kc + 1) * 512) - kc * 512
                            osl = slice(kc * 512, kc * 512 + w)
                            nc.scalar.activation(exp0[:, osl], ec[0][kc][:, 0:w], ACT.Exp,
                                                 scale=ISCALE, accum_out=sA[:, kc:kc + 1])
                            nc.scalar.activation(exp1[:, osl], ec[1][kc][:, 0:w], ACT.Exp,
                                                 scale=ISCALE,
                                                 accum_out=sA[:, nch + kc:nch + kc + 1])
                        if nch == 2:
                            s01 = sm2.tile([128, 2], F32, tag="s01", name="s01")
                            nc.vector.tensor_tensor(
                                s01[:], sA[:, 0:4:2], sA[:, 1:4:2], op=ALU.add)
                        else:
                            s01 = sA
                        r01 = sm2.tile([128, 2], F32, tag="r01", name="r01")
                        nc.vector.reciprocal(r01[:], s01[:, 0:2])
                        # r1p = (s0 * -lam) * r1
                        r1p = sm2.tile([128, 1], F32, tag="r1p", name="r1p")
                        nc.vector.scalar_tensor_tensor(
                            r1p[:], s01[:, 0:1], lam_t[:, h:h + 1], r01[:, 1:2],
                            op0=ALU.mult, op1=ALU.mult)
                        # t = exp1 * r1p + exp0 ; exp2 = Exp(t / s0)
                        t = epool.tile([128, S], BF16, tag="t", name="t")
                        nc.vector.scalar_tensor_tensor(
                            t[:, :Ke], exp1[:, :Ke], r1p[:], exp0[:, :Ke],
                            op0=ALU.mult, op1=ALU.add)
                        exp2 = epool.tile([128, S], BF16, tag="exp2", name="exp2")
                        nc.scalar.activation(exp2[:, :Ke], t[:, :Ke], ACT.Exp,
                                             scale=r01[:, 0:1])
                        # transpose scores into diffT column blocks (PE, bf16);
                        # the diagonal block folds the second causal mask (0/1)
                        isl = slice(i * 128, (i + 1) * 128)
                        for j in range(i + 1):
                            tp = tpps.tile([128, 128], BF16, tag="tp", name="tp")
                            nc.tensor.transpose(
                                tp[:], exp2[:, j * 128:(j + 1) * 128], identb[:])
                            if j == i:
                                nc.vector.tensor_tensor(
                                    diffT[j][:, isl], tp[:], m01_t[:], op=ALU.mult)
                            elif j % 2 == 0:
                                nc.vector.tensor_copy(diffT[j][:, isl], tp[:])
                            else:
                                nc.scalar.copy(diffT[j][:, isl], tp[:])

                    # PV + RMS normalization per 512-q chunk
                    # att_final = att_raw * rsqrt(mean(att^2)+eps); softmax2
                    # normalization cancels inside the RMS.
                    for c in range(NPC):
                        nk = 4 * c + 4
                        attps = pvps.tile([128, 512], F32, tag="att", name="att")
                        for j in range(nk):
                            off = max(0, j * 128 - c * 512)
                            nc.tensor.matmul(
                                attps[:, off:512], v_t[j][:],
                                diffT[j][:, c * 512 + off:(c + 1) * 512],
                                start=(j == 0), stop=(j == nk - 1))
                        atts = sm2.tile([128, 512], BF16, tag="atts", name="atts")
                        nc.scalar.copy(atts[:], attps[:])
                        att2 = sm2.tile([128, 512], F32R, tag="att2", name="att2")
                        nc.scalar.square(att2[:], attps[:])
                        # ssq[1,512] = ones^T @ att2 on the PE (partition reduce)
                        ssq = rmsps.tile([1, 512], F32, tag="ssq", name="ssq")
                        nc.tensor.matmul(
                            ssq[:], ones_c[:], att2[:],
                            start=True, stop=True)
                        # cfac = (mean+eps)^-0.5 = exp(-0.5*ln(ssq/128 + eps))
                        cl = sm2.tile([1, 512], F32, tag="cl", name="cl")
                        nc.scalar.activation(cl[:], ssq[:], ACT.Ln,
                                             scale=1.0 / 128.0, bias=eps_t[:])
                        cf = sm2.tile([1, 512], F32R, tag="cf", name="cf")
                        nc.scalar.activation(cf[:], cl[:], ACT.Exp, scale=-0.5)
                        # broadcast cfac over partitions on the PE
                        bsb = rmsps.tile([128, 512], F32, tag="bsb", name="bsb")
                        nc.tensor.matmul(
                            bsb[:], ones_r[:], cf[:],
                            start=True, stop=True)
                        nc.vector.tensor_tensor(
                            attf[h][c][:], atts[:], bsb[:], op=ALU.mult)

            # ------------- phase C: out = attf^T @ Wo -------------
            with ExitStack() as cctx:
                wops = cctx.enter_context(tc.tile_pool(name="wops", bufs=4, space="PSUM"))
                opool = cctx.enter_context(tc.tile_pool(name="osb", bufs=4))
                for p in range(NQT):
                    c, po = p // 4, (p % 4) * 128
                    for n in range(E // 512):
                        ops = wops.tile([128, 512], F32, tag="o", name="o")
                        for h in range(HPC):
                            nc.tensor.matmul(
                                ops[:], attf[h][c][:, po:po + 128],
                                wo_t[h][:, n * 512:(n + 1) * 512],
                                start=(h == 0), stop=(h == HPC - 1))
                        osb = opool.tile([128, 512], BF16, tag="osb", name="osb")
                        nc.scalar.copy(osb[:], ops[:])
                        nc.sync.dma_start(
                            out=out_ext[p * 128:(p + 1) * 128, n * 512:(n + 1) * 512],
                            in_=osb[:])

    nc.finalize()
    return nc


def _host_prep(x, Wq, Wk, Wv, Wo, lq1, lq2, lk1, lk2, rms_w):
    lam = (np.exp((lq1 * lk1).sum(-1)) - np.exp((lq2 * lk2).sum(-1))
           + LAM_INIT).astype(np.float32)  # (H,)
    j = np.arange(D, dtype=np.float64)
    theta = 1.0 / (10000.0 ** (2.0 * j / (2 * D)))
    pos = np.arange(S, dtype=np.float64)
    ang = pos[None, :] * theta[:, None]  # (128, S)
    cosd = np.cos(ang).astype(np.float32)
    sin = np.sin(ang)
    cosd2 = np.concatenate([np.concatenate([cosd[a * 64:(a + 1) * 64]] * 2, 0)
                            for a in range(2)], 0)
    sind2 = np.concatenate(
        [np.concatenate([-sin[a * 64:(a + 1) * 64], sin[a * 64:(a + 1) * 64]], 0)
         for a in range(2)], 0).astype(np.float32)
    # swap row-halves within each 128-block: device reads sinS at the *input*
    # partition base, writing the product to the swapped output partitions
    sinS = np.concatenate(
        [np.concatenate([sind2[a * 128 + 64:a * 128 + 128],
                         sind2[a * 128:a * 128 + 64]], 0)
         for a in range(2)], 0).astype(np.float32)

    perm256 = np.concatenate([np.arange(0, 128, 2), np.arange(1, 128, 2),
                              np.arange(128, 256, 2), np.arange(129, 256, 2)])
    Wqp = Wq.reshape(E, H, 2 * D)[:, :, perm256].reshape(E, H * 2 * D)
    Wkp = Wk.reshape(E, KVH, 2 * D)[:, :, perm256].reshape(E, KVH * 2 * D)
    WoS = (Wo.reshape(H, D, E) * (rms_w[None, :, None] * (1.0 - LAM_INIT))
           ).reshape(E, E).astype(np.float32)

    qi = np.arange(128)
    maskn = np.where(qi[None, :] > qi[:, None],
                     np.float32(NEG), np.float32(0.0)).astype(np.float32)
    mask01 = (qi[:, None] <= qi[None, :]).astype(np.float32)  # [k,q] k<=q

    import ml_dtypes
    bf = ml_dtypes.bfloat16
    in_maps = []
    for core in range(NCORES):
        b, g = divmod(core, KVH)
        heads = slice(HPC * g * 2 * D, HPC * (g + 1) * 2 * D)
        lam_g = lam[HPC * g:HPC * (g + 1)]
        in_maps.append({
            "xT": np.ascontiguousarray(x[b].T).astype(bf),
            "Wq": np.ascontiguousarray(Wqp[:, heads]).astype(bf),
            "Wk": np.ascontiguousarray(Wkp[:, g * 2 * D:(g + 1) * 2 * D]).astype(bf),
            "Wv": np.ascontiguousarray(Wv[:, g * D:(g + 1) * D]).astype(bf),
            "Wo": np.ascontiguousarray(WoS[HPC * D * g:HPC * D * (g + 1), :]).astype(bf),
            "cosd": cosd2.astype(bf),
            "sind": sinS.astype(bf),
            "lamn": np.tile(-lam_g[None, :], (D, 1)).astype(np.float32),
            "maskn": maskn.astype(bf),
            "mask01": mask01.astype(bf),
        })
    return in_maps


def kernel(x, Wq, Wk, Wv, Wo, lq1, lq2, lk1, lk2, rms_w, _trace=False):
    from concourse import bass_utils

    in_maps = _host_prep(np.asarray(x, np.float32), np.asarray(Wq, np.float32),
                         np.asarray(Wk, np.float32), np.asarray(Wv, np.float32),
                         np.asarray(Wo, np.float32), np.asarray(lq1, np.float32),
                         np.asarray(lq2, np.float32), np.asarray(lk1, np.float32),
                         np.asarray(lk2, np.float32), np.asarray(rms_w, np.float32))
    if "nc" not in _cache:
        _cache["nc"] = _build()
    nc = _cache["nc"]
    res = bass_utils.run_bass_kernel_spmd(
        nc, in_maps, core_ids=list(range(NCORES)), trace=_trace)
    _cache["last_result"] = res
    parts = np.stack([np.asarray(res.results[c]["out"], np.float32)
                      for c in range(NCORES)], 0)
    out = parts.reshape(B, KVH, S, E).sum(1)
    return out.astype(np.float32)


# revision 18
# speedup vs baseline: 3.7531x; 1.0633x over previous
# Differential GQA attention layer (B=2, S=1024, E=2048, H=16, KVH=4, D=128)
# distributed over 8 TRN2 NeuronCores: shard = (batch b, kv-group g) so each
# core owns 1 batch x 4 query heads (1 kv head). All attention is core-local;
# the Wo row-sharded output projection partials are summed on the host.
#
# Self-contained: hardcodes shapes/sharding; builds+compiles a Bass/Tile
# kernel on first call and runs it via run_bass_kernel_spmd on cores 0-7.
import numpy as np

B, S, E, H, KVH = 2, 1024, 2048, 16, 4
D = 128
NEG = -1e30
LAM_INIT = 0.2  # 0.8 - 0.6*exp(-0.3*layer_idx), layer_idx=0
NCORES = 8
HPC = H // KVH  # heads per core = 4

_cache = {}


def _build():
    import concourse.mybir as mybir
    import concourse.tile as tile
    from concourse import bacc
    from concourse.masks import make_identity
    from contextlib import ExitStack

    # Make the act-table pass bind Exp/Ln only to natural_log_exp_and_others
    # (which also has copy/square) so the whole kernel uses ONE table set —
    # each ACT_TABLE_LOAD switch costs ~2.7us and we'd otherwise thrash
    # between exp_and_others and natural_log per RMS block. Set names/order
    # are preserved so act_func_set_id still indexes act_info.json correctly.
    AF = mybir.ActivationFunctionType
    _orig_gat = bacc.get_activation_tables

    def _gat_one_table(arch):
        tabs = _orig_gat(arch)
        out = {}
        for name, funcs in tabs.items():
            if name == "natural_log_exp_and_others":
                out[name] = set(funcs)
            else:
                out[name] = set(funcs) - {AF.Exp, AF.Ln}
        return out

    bacc.get_activation_tables = _gat_one_table

    F32 = mybir.dt.float32
    F32R = mybir.dt.float32r
    BF16 = mybir.dt.bfloat16
    ALU = mybir.AluOpType
    ACT = mybir.ActivationFunctionType

    nc = bacc.Bacc(None, target_bir_lowering=False)

    xT = nc.declare_dram_parameter("xT", [E, S], BF16, isOutput=False)
    Wq = nc.declare_dram_parameter("Wq", [E, HPC * 2 * D], BF16, isOutput=False)
    Wk = nc.declare_dram_parameter("Wk", [E, 2 * D], BF16, isOutput=False)
    Wv = nc.declare_dram_parameter("Wv", [E, D], BF16, isOutput=False)
    Wo = nc.declare_dram_parameter("Wo", [HPC * D, E], BF16, isOutput=False)
    cosd = nc.declare_dram_parameter("cosd", [2 * D, S], BF16, isOutput=False)
    sind = nc.declare_dram_parameter("sind", [2 * D, S], BF16, isOutput=False)
    lamn = nc.declare_dram_parameter("lamn", [D, HPC], F32, isOutput=False)
    maskn = nc.declare_dram_parameter("maskn", [D, D], BF16, isOutput=False)
    mask01 = nc.declare_dram_parameter("mask01", [D, D], BF16, isOutput=False)
    out_ext = nc.declare_dram_parameter("out", [S, E], BF16, isOutput=True)

    ISCALE = 1.0 / float(np.sqrt(D))
    NQT = S // 128
    NPC = S // 512

    with tile.TileContext(nc) as tc:
        with ExitStack() as ctx:
            cpool = ctx.enter_context(tc.tile_pool(name="const", bufs=1))
            wpool = ctx.enter_context(tc.tile_pool(name="wts", bufs=1))
            qkpool = ctx.enter_context(tc.tile_pool(name="qk", bufs=1))
            smalls = ctx.enter_context(tc.tile_pool(name="smalls", bufs=2))

            # ---- weights + x resident in SBUF; full-width rows => 2KB DMA lines
            wk_t = [wpool.tile([128, 2 * D], BF16, tag=f"wk{e}", name=f"wk{e}")
                    for e in range(16)]
            xt = [wpool.tile([128, S], BF16, tag=f"xt{e}", name=f"xt{e}")
                  for e in range(16)]
            wv_t = [wpool.tile([128, D], BF16, tag=f"wv{e}", name=f"wv{e}")
                    for e in range(16)]
            wq_t = [wpool.tile([128, HPC * 2 * D], BF16, tag=f"wq{e}", name=f"wq{e}")
                    for e in range(16)]
            wo_t = [wpool.tile([128, E], BF16, tag=f"wo{h}", name=f"wo{h}")
                    for h in range(HPC)]
            for e in range(16):
                nc.sync.dma_start(out=wk_t[e][:], in_=Wk[e * 128:(e + 1) * 128, :])
                nc.sync.dma_start(out=xt[e][:], in_=xT[e * 128:(e + 1) * 128, :])
            for e in range(16):
                nc.sync.dma_start(out=wv_t[e][:], in_=Wv[e * 128:(e + 1) * 128, :])
            for e in range(16):
                nc.sync.dma_start(out=wq_t[e][:], in_=Wq[e * 128:(e + 1) * 128, :])

            # constants
            cos_t = [cpool.tile([128, S], BF16, tag=f"cos{a}", name=f"cos{a}") for a in range(2)]
            sin_t = [cpool.tile([128, S], BF16, tag=f"sin{a}", name=f"sin{a}") for a in range(2)]
            lam_t = cpool.tile([128, HPC], F32, tag="lam", name="lam")
            mask_t = cpool.tile([128, 128], BF16, tag="mask", name="mask")
            m01_t = cpool.tile([128, 128], BF16, tag="m01", name="m01")
            for a in range(2):
                nc.sync.dma_start(out=cos_t[a][:], in_=cosd[a * 128:(a + 1) * 128, :])
                nc.sync.dma_start(out=sin_t[a][:], in_=sind[a * 128:(a + 1) * 128, :])
            nc.sync.dma_start(out=lam_t[:], in_=lamn[:])
            nc.sync.dma_start(out=mask_t[:], in_=maskn[:])
            nc.sync.dma_start(out=m01_t[:], in_=mask01[:])
            for h in range(HPC):
                nc.sync.dma_start(out=wo_t[h][:], in_=Wo[h * 128:(h + 1) * 128, :])

            identb = cpool.tile([128, 128], BF16, tag="identb", name="identb")
            make_identity(nc, identb[:])
            ones_c32 = cpool.tile([128, 1], F32, tag="ones_c32", name="ones_c32")
            nc.vector.memset(ones_c32[:], 1.0)
            ones_c = cpool.tile([128, 1], F32R, tag="ones_c", name="ones_c")
            nc.scalar.copy(ones_c[:], ones_c32[:])
            ones_r32 = cpool.tile([1, 128], F32, tag="ones_r32", name="ones_r32")
            nc.vector.memset(ones_r32[:], 1.0)
            ones_r = cpool.tile([1, 128], F32R, tag="ones_r", name="ones_r")
            nc.scalar.copy(ones_r[:], ones_r32[:])
            eps_t = cpool.tile([1, 1], F32, tag="eps", name="eps")
            nc.vector.memset(eps_t[:], 1e-8)

            # persistent activations
            qT = [[qkpool.tile([128, S], BF16, tag=f"qT{h}{a}", name=f"qT{h}{a}")
                   for a in range(2)] for h in range(HPC)]
            kT = [qkpool.tile([128, S], BF16, tag=f"kT{a}", name=f"kT{a}") for a in range(2)]
            vT = qkpool.tile([128, S], BF16, tag="vT", name="vT")
            v_t = [qkpool.tile([128, 128], BF16, tag=f"v{j}", name=f"v{j}")
                   for j in range(NQT)]
            attf = [[qkpool.tile([128, 512], BF16, tag=f"attf{h}{c}", name=f"attf{h}{c}")
                     for c in range(NPC)] for h in range(HPC)]

            # ------------- phase A: x @ W -> qT/kT/vT (+rope), v -------------
            with ExitStack() as actx:
                m1ps = actx.enter_context(tc.tile_pool(name="m1ps", bufs=4, space="PSUM"))
                vtrps = actx.enter_context(tc.tile_pool(name="vtrps", bufs=2, space="PSUM"))
                rtmp = actx.enter_context(tc.tile_pool(name="rtmp", bufs=3))

                # k first, then v, then q: phase B head h can start once its q done
                blocks = [("k", 0), ("k", 1), ("v", None)]
                for h in range(HPC):
                    for a in range(2):
                        blocks.append(("q", (h, a)))

                for kind, meta in blocks:
                    pst = [m1ps.tile([128, 512], F32, tag="m1", name="m1")
                           for _ in range(NPC)]
                    for e in range(16):
                        if kind == "k":
                            wsl = wk_t[e][:, meta * 128:(meta + 1) * 128]
                        elif kind == "v":
                            wsl = wv_t[e][:]
                        else:
                            h, a = meta
                            c0 = (h * 2 + a) * 128
                            wsl = wq_t[e][:, c0:c0 + 128]
                        for p in range(NPC):
                            nc.tensor.matmul(
                                pst[p][:], wsl, xt[e][:, p * 512:(p + 1) * 512],
                                start=(e == 0), stop=(e == 15))
                    for p in range(NPC):
                        ps = pst[p]
                        sl = slice(p * 512, (p + 1) * 512)
                        if kind == "v":
                            nc.scalar.copy(vT[:, sl], ps[:])
                            continue
                        a = meta if kind == "k" else meta[1]
                        dst = kT[a] if kind == "k" else qT[meta[0]][a]
                        # rope: dst = c*cos + swap(c)*sin. sin is host-swapped
                        # (sinS[p] = sin[swap(p)]) so each TT reads equal input
                        # bases and only the OUT partition base is shifted.
                        c = rtmp.tile([128, 512], BF16, tag="rc", name="rc")
                        nc.scalar.copy(c[:], ps[:])
                        tmp = rtmp.tile([128, 512], BF16, tag="rt", name="rt")
                        nc.vector.tensor_tensor(
                            tmp[0:64, :], c[64:128, :], sin_t[a][64:128, sl], op=ALU.mult)
                        nc.vector.tensor_tensor(
                            tmp[64:128, :], c[0:64, :], sin_t[a][0:64, sl], op=ALU.mult)
                        nc.vector.tensor_tensor(
                            dst[:, sl], c[:], cos_t[a][:, sl], op=ALU.mult)
                        nc.vector.tensor_tensor(
                            dst[:, sl], dst[:, sl], tmp[:], op=ALU.add)

                # v: [d, pos] -> v_t[j]: [128 pos, 128 d] (bf16 for PV)
                for j in range(NQT):
                    vps = vtrps.tile([128, 128], BF16, tag="vtr", name="vtr")
                    nc.tensor.transpose(
                        vps[:], vT[:, j * 128:(j + 1) * 128], identb[:])
                    nc.scalar.copy(v_t[j][:], vps[:])

            # ------------- phase B: attention per head -------------
            with ExitStack() as bctx:
                eps = bctx.enter_context(tc.tile_pool(name="eps", bufs=3, space="PSUM"))
                tpps = bctx.enter_context(tc.tile_pool(name="tpps", bufs=2, space="PSUM"))
                pvps = bctx.enter_context(tc.tile_pool(name="pvps", bufs=1, space="PSUM"))
                rmsps = bctx.enter_context(tc.tile_pool(name="rmsps", bufs=1, space="PSUM"))
                epool = bctx.enter_context(tc.tile_pool(name="expp", bufs=2))
                dtpool = bctx.enter_context(tc.tile_pool(name="difft", bufs=12))
                sm2 = bctx.enter_context(tc.tile_pool(name="sm2", bufs=3))

                for h in range(HPC):
                    diffT = [dtpool.tile([128, S], BF16, tag="difft", name="difft")
                             for _ in range(NQT)]

                    for i in range(NQT):
                        Ke = (i + 1) * 128
                        nch = 1 if Ke <= 512 else 2
                        dc, doff = (i * 128) // 512, (i * 128) % 512
                        ec = [[eps.tile([128, 512], F32, tag="e", name=f"e{half}")
                               for _ in range(nch)] for half in range(2)]
                        # energy matmuls; diag 128-block gets causal mask added
                        # on the PE (identity @ mask accumulated into psum)
                        for half in range(2):
                            for kc in range(nch):
                                w = min(Ke, (kc + 1) * 512) - kc * 512
                                ksl = slice(kc * 512, kc * 512 + w)
                                qsl = qT[h][half][:, i * 128:(i + 1) * 128]
                                if kc == dc:
                                    if doff > 0:
                                        nc.tensor.matmul(
                                            ec[half][kc][:, 0:doff], qsl,
                                            kT[half][:, kc * 512:kc * 512 + doff],
                                            start=True, stop=True)
                                    nc.tensor.matmul(
                                        ec[half][kc][:, doff:doff + 128], qsl,
                                        kT[half][:, i * 128:i * 128 + 128],
                                        start=True, stop=False)
                                else:
                                    nc.tensor.matmul(
                                        ec[half][kc][:, 0:w], qsl,
                                        kT[half][:, ksl], start=True, stop=True)
                        for half in range(2):
                            nc.tensor.matmul(
                                ec[half][dc][:, doff:doff + 128], identb[:],
                                mask_t[:], start=False, stop=True)

                        exp0 = epool.tile([128, S], BF16, tag="exp0", name="exp0")
                        exp1 = epool.tile([128, S], BF16, tag="exp1", name="exp1")
                        sA = sm2.tile([128, 4], F32, tag="sA", name="sA")
                        for kc in range(nch):
                            w = min(Ke, (kc + 1) * 512) - kc * 512
                            osl = slice(kc * 512, kc * 512 + w)
                            nc.scalar.activation(exp0[:, osl], ec[0][kc][:, 0:w], ACT.Exp,
                                                 scale=ISCALE, accum_out=sA[:, kc:kc + 1])
                            nc.scalar.activation(exp1[:, osl], ec[1][kc][:, 0:w], ACT.Exp,
                                                 scale=ISCALE,
                                                 accum_out=sA[:, nch + kc:nch + kc + 1])
                        if nch == 2:
                            s01 = sm2.tile([128, 2], F32, tag="s01", name="s01")
                            nc.vector.tensor_tensor(
                                s01[:], sA[:, 0:4:2], sA[:, 1:4:2], op=ALU.add)
                        else:
                            s01 = sA
                        r01 = sm2.tile([128, 2], F32, tag="r01", name="r01")
                        nc.vector.reciprocal(r01[:], s01[:, 0:2])
                        # r1p = (s0 * -lam) * r1
                        r1p = sm2.tile([128, 1], F32, tag="r1p", name="r1p")
                        nc.vector.scalar_tensor_tensor(
                            r1p[:], s01[:, 0:1], lam_t[:, h:h + 1], r01[:, 1:2],
                            op0=ALU.mult, op1=ALU.mult)
                        # t = exp1 * r1p + exp0 ; exp2 = Exp(t / s0)
                        t = epool.tile([128, S], BF16, tag="t", name="t")
                        nc.vector.scalar_tensor_tensor(
                            t[:, :Ke], exp1[:, :Ke], r1p[:], exp0[:, :Ke],
                            op0=ALU.mult, op1=ALU.add)
                        exp2 = epool.tile([128, S], BF16, tag="exp2", name="exp2")
                        nc.scalar.activation(exp2[:, :Ke], t[:, :Ke], ACT.Exp,
                                             scale=r01[:, 0:1])
                        # transpose scores into diffT column blocks (PE, bf16);
                        # the diagonal block folds the second causal mask (0/1)
                        isl = slice(i * 128, (i + 1) * 128)
                        for j in range(i + 1):
                            tp = tpps.tile([128, 128], BF16, tag="tp", name="tp")
                            nc.tensor.transpose(
                                tp[:], exp2[:, j * 128:(j + 1) * 128], identb[:])
                            if j == i:
                                nc.vector.tensor_tensor(
                                    diffT[j][:, isl], tp[:], m01_t[:], op=ALU.mult)
                            elif j % 2 == 0:
                                nc.vector.tensor_copy(diffT[j][:, isl], tp[:])
                            else:
                                nc.scalar.copy(diffT[j][:, isl], tp[:])

                    # PV + RMS normalization per 512-q chunk
                    # att_final = att_raw * rsqrt(mean(att^2)+eps); softmax2
                    # normalization cancels inside the RMS.
                    for c in range(NPC):
                        nk = 4 * c + 4
                        attps = pvps.tile([128, 512], F32, tag="att", name="att")
                        for j in range(nk):
                            off = max(0, j * 128 - c * 512)
                            nc.tensor.matmul(
                                attps[:, off:512], v_t[j][:],
                                diffT[j][:, c * 512 + off:(c + 1) * 512],
                                start=(j == 0), stop=(j == nk - 1))
                        atts = sm2.tile([128, 512], BF16, tag="atts", name="atts")
                        nc.scalar.copy(atts[:], attps[:])
                        att2 = sm2.tile([128, 512], F32R, tag="att2", name="att2")
                        nc.scalar.square(att2[:], attps[:])
                        # ssq[1,512] = ones^T @ att2 on the PE (partition reduce)
                        ssq = rmsps.tile([1, 512], F32, tag="ssq", name="ssq")
                        nc.tensor.matmul(
                            ssq[:], ones_c[:], att2[:],
                            start=True, stop=True)
                        # cfac = (mean+eps)^-0.5 = exp(-0.5*ln(ssq/128 + eps))
                        cl = sm2.tile([1, 512], F32, tag="cl", name="cl")
                        nc.scalar.activation(cl[:], ssq[:], ACT.Ln,
                                             scale=1.0 / 128.0, bias=eps_t[:])
                        cf = sm2.tile([1, 512], F32R, tag="cf", name="cf")
                        nc.scalar.activation(cf[:], cl[:], ACT.Exp, scale=-0.5)
                        # broadcast cfac over partitions on the PE
                        bsb = rmsps.tile([128, 512], F32, tag="bsb", name="bsb")
                        nc.tensor.matmul(
                            bsb[:], ones_r[:], cf[:],
                            start=True, stop=True)
                        nc.vector.tensor_tensor(
                            attf[h][c][:], atts[:], bsb[:], op=ALU.mult)

            # ------------- phase C: out = attf^T @ Wo -------------
            with ExitStack() as cctx:
                wops = cctx.enter_context(tc.tile_pool(name="wops", bufs=4, space="PSUM"))
                opool = cctx.enter_context(tc.tile_pool(name="osb", bufs=4))
                for p in range(NQT):
                    c, po = p // 4, (p % 4) * 128
                    for n in range(E // 512):
                        ops = wops.tile([128, 512], F32, tag="o", name="o")
                        for h in range(HPC):
                            nc.tensor.matmul(
                                ops[:], attf[h][c][:, po:po + 128],
                                wo_t[h][:, n * 512:(n + 1) * 512],
                                start=(h == 0), stop=(h == HPC - 1))
                        osb = opool.tile([128, 512], BF16, tag="osb", name="osb")
                        nc.vector.tensor_copy(osb[:], ops[:])
                        nc.sync.dma_start(
                            out=out_ext[p * 128:(p + 1) * 128, n * 512:(n + 1) * 512],
                            in_=osb[:])

    nc.finalize()
    bacc.get_activation_tables = _orig_gat
    return nc


def _host_prep(x, Wq, Wk, Wv, Wo, lq1, lq2, lk1, lk2, rms_w):
    lam = (np.exp((lq1 * lk1).sum(-1)) - np.exp((lq2 * lk2).sum(-1))
           + LAM_INIT).astype(np.float32)  # (H,)
    j = np.arange(D, dtype=np.float64)
    theta = 1.0 / (10000.0 ** (2.0 * j / (2 * D)))
    pos = np.arange(S, dtype=np.float64)
    ang = pos[None, :] * theta[:, None]  # (128, S)
    cosd = np.cos(ang).astype(np.float32)
    sin = np.sin(ang)
    cosd2 = np.concatenate([np.concatenate([cosd[a * 64:(a + 1) * 64]] * 2, 0)
                            for a in range(2)], 0)
    sind2 = np.concatenate(
        [np.concatenate([-sin[a * 64:(a + 1) * 64], sin[a * 64:(a + 1) * 64]], 0)
         for a in range(2)], 0).astype(np.float32)
    # swap row-halves within each 128-block: device reads sinS at the *input*
    # partition base, writing the product to the swapped output partitions
    sinS = np.concatenate(
        [np.concatenate([sind2[a * 128 + 64:a * 128 + 128],
                         sind2[a * 128:a * 128 + 64]], 0)
         for a in range(2)], 0).astype(np.float32)

    perm256 = np.concatenate([np.arange(0, 128, 2), np.arange(1, 128, 2),
                              np.arange(128, 256, 2), np.arange(129, 256, 2)])
    Wqp = Wq.reshape(E, H, 2 * D)[:, :, perm256].reshape(E, H * 2 * D)
    Wkp = Wk.reshape(E, KVH, 2 * D)[:, :, perm256].reshape(E, KVH * 2 * D)
    WoS = (Wo.reshape(H, D, E) * (rms_w[None, :, None] * (1.0 - LAM_INIT))
           ).reshape(E, E).astype(np.float32)

    qi = np.arange(128)
    maskn = np.where(qi[None, :] > qi[:, None],
                     np.float32(NEG), np.float32(0.0)).astype(np.float32)
    mask01 = (qi[:, None] <= qi[None, :]).astype(np.float32)  # [k,q] k<=q

    import ml_dtypes
    bf = ml_dtypes.bfloat16
    in_maps = []
    for core in range(NCORES):
        b, g = divmod(core, KVH)
        heads = slice(HPC * g * 2 * D, HPC * (g + 1) * 2 * D)
        lam_g = lam[HPC * g:HPC * (g + 1)]
        in_maps.append({
            "xT": np.ascontiguousarray(x[b].T).astype(bf),
            "Wq": np.ascontiguousarray(Wqp[:, heads]).astype(bf),
            "Wk": np.ascontiguousarray(Wkp[:, g * 2 * D:(g + 1) * 2 * D]).astype(bf),
            "Wv": np.ascontiguousarray(Wv[:, g * D:(g + 1) * D]).astype(bf),
            "Wo": np.ascontiguousarray(WoS[HPC * D * g:HPC * D * (g + 1), :]).astype(bf),
            "cosd": cosd2.astype(bf),
            "sind": sinS.astype(bf),
            "lamn": np.tile(-lam_g[None, :], (D, 1)).astype(np.float32),
            "maskn": maskn.astype(bf),
            "mask01": mask01.astype(bf),
        })
    return in_maps


def kernel(x, Wq, Wk, Wv, Wo, lq1, lq2, lk1, lk2, rms_w, _trace=False):
    from concourse import bass_utils

    in_maps = _host_prep(np.asarray(x, np.float32), np.asarray(Wq, np.float32),
                         np.asarray(Wk, np.float32), np.asarray(Wv, np.float32),
                         np.asarray(Wo, np.float32), np.asarray(lq1, np.float32),
                         np.asarray(lq2, np.float32), np.asarray(lk1, np.float32),
                         np.asarray(lk2, np.float32), np.asarray(rms_w, np.float32))
    if "nc" not in _cache:
        _cache["nc"] = _build()
    nc = _cache["nc"]
    res = bass_utils.run_bass_kernel_spmd(
        nc, in_maps, core_ids=list(range(NCORES)), trace=_trace)
    _cache["last_result"] = res
    parts = np.stack([np.asarray(res.results[c]["out"], np.float32)
                      for c in range(NCORES)], 0)
    out = parts.reshape(B, KVH, S, E).sum(1)
    return out.astype(np.float32)


# revision 21
# speedup vs baseline: 3.8923x; 1.0371x over previous
# Differential GQA attention layer (B=2, S=1024, E=2048, H=16, KVH=4, D=128)
# distributed over 8 TRN2 NeuronCores: shard = (batch b, kv-group g) so each
# core owns 1 batch x 4 query heads (1 kv head). All attention is core-local;
# the Wo row-sharded output projection partials are summed on the host.
#
# Self-contained: hardcodes shapes/sharding; builds+compiles a Bass/Tile
# kernel on first call and runs it via run_bass_kernel_spmd on cores 0-7.
import numpy as np

B, S, E, H, KVH = 2, 1024, 2048, 16, 4
D = 128
NEG = -1e30
LAM_INIT = 0.2  # 0.8 - 0.6*exp(-0.3*layer_idx), layer_idx=0
NCORES = 8
HPC = H // KVH  # heads per core = 4

_cache = {}


def _build():
    import concourse.mybir as mybir
    import concourse.tile as tile
    from concourse import bacc
    from concourse.masks import make_identity
    from contextlib import ExitStack

    # Make the act-table pass bind Exp/Ln only to natural_log_exp_and_others
    # (which also has copy/square) so the whole kernel uses ONE table set —
    # each ACT_TABLE_LOAD switch costs ~2.7us and we'd otherwise thrash
    # between exp_and_others and natural_log per RMS block. Set names/order
    # are preserved so act_func_set_id still indexes act_info.json correctly.
    AF = mybir.ActivationFunctionType
    _orig_gat = bacc.get_activation_tables

    def _gat_one_table(arch):
        tabs = _orig_gat(arch)
        out = {}
        for name, funcs in tabs.items():
            if name == "natural_log_exp_and_others":
                out[name] = set(funcs)
            else:
                out[name] = set(funcs) - {AF.Exp, AF.Ln}
        return out

    bacc.get_activation_tables = _gat_one_table

    F32 = mybir.dt.float32
    F32R = mybir.dt.float32r
    BF16 = mybir.dt.bfloat16
    ALU = mybir.AluOpType
    ACT = mybir.ActivationFunctionType

    nc = bacc.Bacc(None, target_bir_lowering=False)

    xT = nc.declare_dram_parameter("xT", [E, S], BF16, isOutput=False)
    Wq = nc.declare_dram_parameter("Wq", [E, HPC * 2 * D], BF16, isOutput=False)
    Wk = nc.declare_dram_parameter("Wk", [E, 2 * D], BF16, isOutput=False)
    Wv = nc.declare_dram_parameter("Wv", [E, D], BF16, isOutput=False)
    Wo = nc.declare_dram_parameter("Wo", [HPC * D, E], BF16, isOutput=False)
    cosd = nc.declare_dram_parameter("cosd", [2 * D, S], BF16, isOutput=False)
    sind = nc.declare_dram_parameter("sind", [2 * D, S], BF16, isOutput=False)
    lamn = nc.declare_dram_parameter("lamn", [D, HPC], F32, isOutput=False)
    maskn = nc.declare_dram_parameter("maskn", [D, D], BF16, isOutput=False)
    mask01 = nc.declare_dram_parameter("mask01", [D, D], BF16, isOutput=False)
    out_ext = nc.declare_dram_parameter("out", [S, E], BF16, isOutput=True)

    ISCALE = 1.0 / float(np.sqrt(D))
    NQT = S // 128
    NPC = S // 512

    with tile.TileContext(nc) as tc:
        with ExitStack() as ctx:
            cpool = ctx.enter_context(tc.tile_pool(name="const", bufs=1))
            wpool = ctx.enter_context(tc.tile_pool(name="wts", bufs=1))
            qkpool = ctx.enter_context(tc.tile_pool(name="qk", bufs=1))
            smalls = ctx.enter_context(tc.tile_pool(name="smalls", bufs=2))

            # ---- weights + x resident in SBUF; full-width rows => 2KB DMA lines
            wk_t = [wpool.tile([128, 2 * D], BF16, tag=f"wk{e}", name=f"wk{e}")
                    for e in range(16)]
            xt = [wpool.tile([128, S], BF16, tag=f"xt{e}", name=f"xt{e}")
                  for e in range(16)]
            wv_t = [wpool.tile([128, D], BF16, tag=f"wv{e}", name=f"wv{e}")
                    for e in range(16)]
            wq_t = [wpool.tile([128, HPC * 2 * D], BF16, tag=f"wq{e}", name=f"wq{e}")
                    for e in range(16)]
            wo_t = [wpool.tile([128, E], BF16, tag=f"wo{h}", name=f"wo{h}")
                    for h in range(HPC)]
            for e in range(16):
                nc.sync.dma_start(out=wk_t[e][:], in_=Wk[e * 128:(e + 1) * 128, :])
                nc.sync.dma_start(out=xt[e][:], in_=xT[e * 128:(e + 1) * 128, :])
            for e in range(16):
                nc.sync.dma_start(out=wv_t[e][:], in_=Wv[e * 128:(e + 1) * 128, :])
            for e in range(16):
                nc.sync.dma_start(out=wq_t[e][:], in_=Wq[e * 128:(e + 1) * 128, :])

            # constants
            cos_t = [cpool.tile([128, S], BF16, tag=f"cos{a}", name=f"cos{a}") for a in range(2)]
            sin_t = [cpool.tile([128, S], BF16, tag=f"sin{a}", name=f"sin{a}") for a in range(2)]
            lam_t = cpool.tile([128, HPC], F32, tag="lam", name="lam")
            mask_t = cpool.tile([128, 128], BF16, tag="mask", name="mask")
            m01_t = cpool.tile([128, 128], BF16, tag="m01", name="m01")
            for a in range(2):
                nc.sync.dma_start(out=cos_t[a][:], in_=cosd[a * 128:(a + 1) * 128, :])
                nc.sync.dma_start(out=sin_t[a][:], in_=sind[a * 128:(a + 1) * 128, :])
            nc.sync.dma_start(out=lam_t[:], in_=lamn[:])
            nc.sync.dma_start(out=mask_t[:], in_=maskn[:])
            nc.sync.dma_start(out=m01_t[:], in_=mask01[:])
            for h in range(HPC):
                nc.sync.dma_start(out=wo_t[h][:], in_=Wo[h * 128:(h + 1) * 128, :])

            identb = cpool.tile([128, 128], BF16, tag="identb", name="identb")
            make_identity(nc, identb[:])
            ones_c32 = cpool.tile([128, 1], F32, tag="ones_c32", name="ones_c32")
            nc.vector.memset(ones_c32[:], 1.0)
            ones_c = cpool.tile([128, 1], F32R, tag="ones_c", name="ones_c")
            nc.scalar.copy(ones_c[:], ones_c32[:])
            ones_r32 = cpool.tile([1, 128], F32, tag="ones_r32", name="ones_r32")
            nc.vector.memset(ones_r32[:], 1.0)
            ones_r = cpool.tile([1, 128], F32R, tag="ones_r", name="ones_r")
            nc.scalar.copy(ones_r[:], ones_r32[:])
            eps_t = cpool.tile([1, 1], F32, tag="eps", name="eps")
            nc.vector.memset(eps_t[:], 1e-8)

            # persistent activations
            qT = [[qkpool.tile([128, S], BF16, tag=f"qT{h}{a}", name=f"qT{h}{a}")
                   for a in range(2)] for h in range(HPC)]
            kT = [qkpool.tile([128, S], BF16, tag=f"kT{a}", name=f"kT{a}") for a in range(2)]
            vT = qkpool.tile([128, S], BF16, tag="vT", name="vT")
            v_t = [qkpool.tile([128, 128], BF16, tag=f"v{j}", name=f"v{j}")
                   for j in range(NQT)]
            attf = [[qkpool.tile([128, 512], BF16, tag=f"attf{h}{c}", name=f"attf{h}{c}")
                     for c in range(NPC)] for h in range(HPC)]

            # ------------- phase A: x @ W -> qT/kT/vT (+rope), v -------------
            with ExitStack() as actx:
                m1ps = actx.enter_context(tc.tile_pool(name="m1ps", bufs=4, space="PSUM"))
                vtrps = actx.enter_context(tc.tile_pool(name="vtrps", bufs=2, space="PSUM"))
                rtmp = actx.enter_context(tc.tile_pool(name="rtmp", bufs=3))

                # k first, then v, then q: phase B head h can start once its q done
                blocks = [("k", 0), ("k", 1), ("v", None)]
                for h in range(HPC):
                    for a in range(2):
                        blocks.append(("q", (h, a)))

                for kind, meta in blocks:
                    pst = [m1ps.tile([128, 512], F32, tag="m1", name="m1")
                           for _ in range(NPC)]
                    for e in range(16):
                        if kind == "k":
                            wsl = wk_t[e][:, meta * 128:(meta + 1) * 128]
                        elif kind == "v":
                            wsl = wv_t[e][:]
                        else:
                            h, a = meta
                            c0 = (h * 2 + a) * 128
                            wsl = wq_t[e][:, c0:c0 + 128]
                        for p in range(NPC):
                            nc.tensor.matmul(
                                pst[p][:], wsl, xt[e][:, p * 512:(p + 1) * 512],
                                start=(e == 0), stop=(e == 15))
                    for p in range(NPC):
                        ps = pst[p]
                        sl = slice(p * 512, (p + 1) * 512)
                        if kind == "v":
                            nc.scalar.copy(vT[:, sl], ps[:])
                            continue
                        a = meta if kind == "k" else meta[1]
                        dst = kT[a] if kind == "k" else qT[meta[0]][a]
                        # rope: dst = c*cos + swap(c)*sin. sin is host-swapped
                        # (sinS[p] = sin[swap(p)]) so each TT reads equal input
                        # bases and only the OUT partition base is shifted.
                        c = rtmp.tile([128, 512], BF16, tag="rc", name="rc")
                        nc.scalar.copy(c[:], ps[:])
                        tmp = rtmp.tile([128, 512], BF16, tag="rt", name="rt")
                        nc.vector.tensor_tensor(
                            tmp[0:64, :], c[64:128, :], sin_t[a][64:128, sl], op=ALU.mult)
                        nc.vector.tensor_tensor(
                            tmp[64:128, :], c[0:64, :], sin_t[a][0:64, sl], op=ALU.mult)
                        nc.vector.tensor_tensor(
                            dst[:, sl], c[:], cos_t[a][:, sl], op=ALU.mult)
                        nc.vector.tensor_tensor(
                            dst[:, sl], dst[:, sl], tmp[:], op=ALU.add)

                # v: [d, pos] -> v_t[j]: [128 pos, 128 d] (bf16 for PV)
                for j in range(NQT):
                    vps = vtrps.tile([128, 128], BF16, tag="vtr", name="vtr")
                    nc.tensor.transpose(
                        vps[:], vT[:, j * 128:(j + 1) * 128], identb[:])
                    nc.scalar.copy(v_t[j][:], vps[:])

            # ------------- phase B: attention per head -------------
            with ExitStack() as bctx:
                eps = bctx.enter_context(tc.tile_pool(name="eps", bufs=3, space="PSUM"))
                tpps = bctx.enter_context(tc.tile_pool(name="tpps", bufs=2, space="PSUM"))
                pvps = bctx.enter_context(tc.tile_pool(name="pvps", bufs=1, space="PSUM"))
                rmsps = bctx.enter_context(tc.tile_pool(name="rmsps", bufs=1, space="PSUM"))
                epool = bctx.enter_context(tc.tile_pool(name="expp", bufs=2))
                dtpool = bctx.enter_context(tc.tile_pool(name="difft", bufs=16))
                sm2 = bctx.enter_context(tc.tile_pool(name="sm2", bufs=3))

                def pv_rms(h, c, diffT):
                    nk = 4 * c + 4
                    attps = pvps.tile([128, 512], F32, tag="att", name="att")
                    for j in range(nk):
                        off = max(0, j * 128 - c * 512)
                        nc.tensor.matmul(
                            attps[:, off:512], v_t[j][:],
                            diffT[j][:, c * 512 + off:(c + 1) * 512],
                            start=(j == 0), stop=(j == nk - 1))
                    atts = sm2.tile([128, 512], BF16, tag="atts", name="atts")
                    nc.scalar.copy(atts[:], attps[:])
                    att2 = sm2.tile([128, 512], F32R, tag="att2", name="att2")
                    nc.scalar.square(att2[:], attps[:])
                    # ssq[1,512] = ones^T @ att2 on the PE (partition reduce)
                    ssq = rmsps.tile([1, 512], F32, tag="ssq", name="ssq")
                    nc.tensor.matmul(ssq[:], ones_c[:], att2[:],
                                     start=True, stop=True)
                    # cfac = (mean+eps)^-0.5 = exp(-0.5*ln(ssq/128 + eps))
                    cl = sm2.tile([1, 512], F32, tag="cl", name="cl")
                    nc.scalar.activation(cl[:], ssq[:], ACT.Ln,
                                         scale=1.0 / 128.0, bias=eps_t[:])
                    cf = sm2.tile([1, 512], F32R, tag="cf", name="cf")
                    nc.scalar.activation(cf[:], cl[:], ACT.Exp, scale=-0.5)
                    # broadcast cfac over partitions on the PE
                    bsb = rmsps.tile([128, 512], F32, tag="bsb", name="bsb")
                    nc.tensor.matmul(bsb[:], ones_r[:], cf[:],
                                     start=True, stop=True)
                    nc.vector.tensor_tensor(
                        attf[h][c][:], atts[:], bsb[:], op=ALU.mult)

                # heads processed in interleaved pairs: while one head's
                # softmax chain runs on Act/DVE, the other's matmuls keep
                # the PE busy (HAM stays un-throttled)
                for hp in range(HPC // 2):
                    diffTs = [[dtpool.tile([128, S], BF16, tag="difft", name="difft")
                               for _ in range(NQT)] for _ in range(2)]

                    for i in range(NQT):
                      for hh in range(2):
                        h = 2 * hp + hh
                        diffT = diffTs[hh]
                        Ke = (i + 1) * 128
                        nch = 1 if Ke <= 512 else 2
                        dc, doff = (i * 128) // 512, (i * 128) % 512
                        ec = [[eps.tile([128, 512], F32, tag="e", name=f"e{half}")
                               for _ in range(nch)] for half in range(2)]
                        # energy matmuls; diag 128-block gets causal mask added
                        # on the PE (identity @ mask accumulated into psum)
                        for half in range(2):
                            for kc in range(nch):
                                w = min(Ke, (kc + 1) * 512) - kc * 512
                                ksl = slice(kc * 512, kc * 512 + w)
                                qsl = qT[h][half][:, i * 128:(i + 1) * 128]
                                if kc == dc:
                                    if doff > 0:
                                        nc.tensor.matmul(
                                            ec[half][kc][:, 0:doff], qsl,
                                            kT[half][:, kc * 512:kc * 512 + doff],
                                            start=True, stop=True)
                                    nc.tensor.matmul(
                                        ec[half][kc][:, doff:doff + 128], qsl,
                                        kT[half][:, i * 128:i * 128 + 128],
                                        start=True, stop=False)
                                else:
                                    nc.tensor.matmul(
                                        ec[half][kc][:, 0:w], qsl,
                                        kT[half][:, ksl], start=True, stop=True)
                        for half in range(2):
                            nc.tensor.matmul(
                                ec[half][dc][:, doff:doff + 128], identb[:],
                                mask_t[:], start=False, stop=True)

                        exp0 = epool.tile([128, S], BF16, tag="exp0", name="exp0")
                        exp1 = epool.tile([128, S], BF16, tag="exp1", name="exp1")
                        sA = sm2.tile([128, 4], F32, tag="sA", name="sA")
                        for kc in range(nch):
                            w = min(Ke, (kc + 1) * 512) - kc * 512
                            osl = slice(kc * 512, kc * 512 + w)
                            nc.scalar.activation(exp0[:, osl], ec[0][kc][:, 0:w], ACT.Exp,
                                                 scale=ISCALE, accum_out=sA[:, kc:kc + 1])
                            nc.scalar.activation(exp1[:, osl], ec[1][kc][:, 0:w], ACT.Exp,
                                                 scale=ISCALE,
                                                 accum_out=sA[:, nch + kc:nch + kc + 1])
                        if nch == 2:
                            s01 = sm2.tile([128, 2], F32, tag="s01", name="s01")
                            nc.vector.tensor_tensor(
                                s01[:], sA[:, 0:4:2], sA[:, 1:4:2], op=ALU.add)
                        else:
                            s01 = sA
                        r01 = sm2.tile([128, 2], F32, tag="r01", name="r01")
                        nc.vector.reciprocal(r01[:], s01[:, 0:2])
                        # r1p = (s0 * -lam) * r1
                        r1p = sm2.tile([128, 1], F32, tag="r1p", name="r1p")
                        nc.vector.scalar_tensor_tensor(
                            r1p[:], s01[:, 0:1], lam_t[:, h:h + 1], r01[:, 1:2],
                            op0=ALU.mult, op1=ALU.mult)
                        # t = exp1 * r1p + exp0 ; exp2 = Exp(t / s0)
                        t = epool.tile([128, S], BF16, tag="t", name="t")
                        nc.vector.scalar_tensor_tensor(
                            t[:, :Ke], exp1[:, :Ke], r1p[:], exp0[:, :Ke],
                            op0=ALU.mult, op1=ALU.add)
                        exp2 = epool.tile([128, S], BF16, tag="exp2", name="exp2")
                        nc.scalar.activation(exp2[:, :Ke], t[:, :Ke], ACT.Exp,
                                             scale=r01[:, 0:1])
                        # transpose scores into diffT column blocks (PE, bf16);
                        # the diagonal block folds the second causal mask (0/1)
                        isl = slice(i * 128, (i + 1) * 128)
                        for j in range(i + 1):
                            tp = tpps.tile([128, 128], BF16, tag="tp", name="tp")
                            nc.tensor.transpose(
                                tp[:], exp2[:, j * 128:(j + 1) * 128], identb[:])
                            if j == i:
                                nc.vector.tensor_tensor(
                                    diffT[j][:, isl], tp[:], m01_t[:], op=ALU.mult)
                            else:
                                nc.vector.tensor_copy(diffT[j][:, isl], tp[:])
                      if i == 3:
                        # first q-chunk's PV+RMS is ready: extra PE/Act work
                        # to fill the bubbles while later i's softmax runs
                        for hh in range(2):
                            pv_rms(2 * hp + hh, 0, diffTs[hh])

                    for hh in range(2):
                        pv_rms(2 * hp + hh, 1, diffTs[hh])

            # ------------- phase C: out = attf^T @ Wo -------------
            with ExitStack() as cctx:
                wops = cctx.enter_context(tc.tile_pool(name="wops", bufs=4, space="PSUM"))
                opool = cctx.enter_context(tc.tile_pool(name="osb", bufs=4))
                for p in range(NQT):
                    c, po = p // 4, (p % 4) * 128
                    for n in range(E // 512):
                        ops = wops.tile([128, 512], F32, tag="o", name="o")
                        for h in range(HPC):
                            nc.tensor.matmul(
                                ops[:], attf[h][c][:, po:po + 128],
                                wo_t[h][:, n * 512:(n + 1) * 512],
                                start=(h == 0), stop=(h == HPC - 1))
                        osb = opool.tile([128, 512], BF16, tag="osb", name="osb")
                        nc.vector.tensor_copy(osb[:], ops[:])
                        nc.sync.dma_start(
                            out=out_ext[p * 128:(p + 1) * 128, n * 512:(n + 1) * 512],
                            in_=osb[:])

    nc.finalize()
    bacc.get_activation_tables = _orig_gat
    return nc


def _host_prep(x, Wq, Wk, Wv, Wo, lq1, lq2, lk1, lk2, rms_w):
    lam = (np.exp((lq1 * lk1).sum(-1)) - np.exp((lq2 * lk2).sum(-1))
           + LAM_INIT).astype(np.float32)  # (H,)
    j = np.arange(D, dtype=np.float64)
    theta = 1.0 / (10000.0 ** (2.0 * j / (2 * D)))
    pos = np.arange(S, dtype=np.float64)
    ang = pos[None, :] * theta[:, None]  # (128, S)
    cosd = np.cos(ang).astype(np.float32)
    sin = np.sin(ang)
    cosd2 = np.concatenate([np.concatenate([cosd[a * 64:(a + 1) * 64]] * 2, 0)
                            for a in range(2)], 0)
    sind2 = np.concatenate(
        [np.concatenate([-sin[a * 64:(a + 1) * 64], sin[a * 64:(a + 1) * 64]], 0)
         for a in range(2)], 0).astype(np.float32)
    # swap row-halves within each 128-block: device reads sinS at the *input*
    # partition base, writing the product to the swapped output partitions
    sinS = np.concatenate(
        [np.concatenate([sind2[a * 128 + 64:a * 128 + 128],
                         sind2[a * 128:a * 128 + 64]], 0)
         for a in range(2)], 0).astype(np.float32)

    perm256 = np.concatenate([np.arange(0, 128, 2), np.arange(1, 128, 2),
                              np.arange(128, 256, 2), np.arange(129, 256, 2)])
    Wqp = Wq.reshape(E, H, 2 * D)[:, :, perm256].reshape(E, H * 2 * D)
    Wkp = Wk.reshape(E, KVH, 2 * D)[:, :, perm256].reshape(E, KVH * 2 * D)
    WoS = (Wo.reshape(H, D, E) * (rms_w[None, :, None] * (1.0 - LAM_INIT))
           ).reshape(E, E).astype(np.float32)

    qi = np.arange(128)
    maskn = np.where(qi[None, :] > qi[:, None],
                     np.float32(NEG), np.float32(0.0)).astype(np.float32)
    mask01 = (qi[:, None] <= qi[None, :]).astype(np.float32)  # [k,q] k<=q

    import ml_dtypes
    bf = ml_dtypes.bfloat16
    in_maps = []
    for core in range(NCORES):
        b, g = divmod(core, KVH)
        heads = slice(HPC * g * 2 * D, HPC * (g + 1) * 2 * D)
        lam_g = lam[HPC * g:HPC * (g + 1)]
        in_maps.append({
            "xT": np.ascontiguousarray(x[b].T).astype(bf),
            "Wq": np.ascontiguousarray(Wqp[:, heads]).astype(bf),
            "Wk": np.ascontiguousarray(Wkp[:, g * 2 * D:(g + 1) * 2 * D]).astype(bf),
            "Wv": np.ascontiguousarray(Wv[:, g * D:(g + 1) * D]).astype(bf),
            "Wo": np.ascontiguousarray(WoS[HPC * D * g:HPC * D * (g + 1), :]).astype(bf),
            "cosd": cosd2.astype(bf),
            "sind": sinS.astype(bf),
            "lamn": np.tile(-lam_g[None, :], (D, 1)).astype(np.float32),
            "maskn": maskn.astype(bf),
            "mask01": mask01.astype(bf),
        })
    return in_maps


def kernel(x, Wq, Wk, Wv, Wo, lq1, lq2, lk1, lk2, rms_w, _trace=False):
    from concourse import bass_utils

    in_maps = _host_prep(np.asarray(x, np.float32), np.asarray(Wq, np.float32),
                         np.asarray(Wk, np.float32), np.asarray(Wv, np.float32),
                         np.asarray(Wo, np.float32), np.asarray(lq1, np.float32),
                         np.asarray(lq2, np.float32), np.asarray(lk1, np.float32),
                         np.asarray(lk2, np.float32), np.asarray(rms_w, np.float32))
    if "nc" not in _cache:
        _cache["nc"] = _build()
    nc = _cache["nc"]
    res = bass_utils.run_bass_kernel_spmd(
        nc, in_maps, core_ids=list(range(NCORES)), trace=_trace)
    _cache["last_result"] = res
    parts = np.stack([np.asarray(res.results[c]["out"], np.float32)
                      for c in range(NCORES)], 0)
    out = parts.reshape(B, KVH, S, E).sum(1)
    return out.astype(np.float32)
